# revision 43
# baseline (speedup 1.0000x reference)
"""Distributed multi-head attention kernel for 8 TRN2 NeuronCores (v2).

Problem: x(4,2048,1024) -> qkv proj (w_qkv 3072x1024) -> 16-head attention
(head_dim 64, softmax) -> out proj (w_out 1024x1024 + b_out).

Sharding: head-parallel. Core c owns heads {2c, 2c+1}; per-batch AllToAll
converts head-sharded attention output to token-sharded for the out proj.

v2 schedule: the attention phase is ScalarE(exp)-bound, so all projection
work is spread through it as PE "filler" units instead of lumping per
batch. PSUM layout (8 banks): e3 [128,1536] (3) + e2 [128,1024] (2) for
double-buffered exp tiles (groups alternate widths 3/2), pv 2x[128,512]
(2) for the PV accumulators, fill [128,512] (1) for filler matmul units
(QKV pieces of batch b+1 and out-proj halves of batch b-1, one unit every
other attention group). Batch 0's QKV runs as a prologue on the e3/e2/fill
rotation with qt0 attention interleaved; batch 3's out-proj m=0 half runs
as qt3 filler while its A2A pieces (qt0+qt1 half at qt2, qt2 quarter at
qt3, qt3 quarter at the end) keep the tail collective small.

Measured on 8 axon-tunneled trn2 cores: ~460-464 us HW exec (traced),
rel err 5.2e-3, vs ~524 us for the previous per-batch-phase schedule.
"""

import numpy as np
import ml_dtypes

import concourse.bass as bass
import concourse.mybir as mybir
import concourse.tile as tile
from concourse import bacc, bass_utils
from concourse.tile import add_dep_helper

FP32 = mybir.dt.float32
BF16 = mybir.dt.bfloat16
AF = mybir.ActivationFunctionType

N_CORES = 8
B, NTOK, D = 4, 2048, 1024
T = B * NTOK  # 8192 tokens total
NH, HD = 16, 64
HL = NH // N_CORES  # 2 heads per core
SCALE = float(HD) ** -0.5  # 0.125
TN = 512  # token tile for QKV / q tile for attention
NT = T // TN  # 16
KC = D // 128  # 8 contraction chunks for projections
KT = NTOK // 128  # 16 k-chunks per batch in attention
TPB = NTOK // N_CORES  # 256 tokens per (core, batch) after A2A
TPC = T // N_CORES  # 1024 tokens per core total
WCOLS = 3 * HL * HD  # 384 qkv output dims per core

# attention group widths per q-tile: 13 groups, 3/2 alternating (32 slots)
WIDTHS = [3, 2] * 6 + [2]


def build_nc(debug=False):
    nc = bacc.Bacc(
        "TRN2", target_bir_lowering=False, debug=False, num_devices=N_CORES
    )
    xt = nc.dram_tensor("xt", [D, T], BF16, kind="ExternalInput").ap()
    wt = nc.dram_tensor("wt", [D, WCOLS], BF16, kind="ExternalInput").ap()
    wo = nc.dram_tensor("wo", [D, D], BF16, kind="ExternalInput").ap()
    bias = nc.dram_tensor("bias", [1, D], FP32, kind="ExternalInput").ap()
    out = nc.dram_tensor("out", [TPC, D], FP32, kind="ExternalOutput").ap()

    xt3 = xt.rearrange("(kc p) j -> p kc j", p=128)

    with tile.TileContext(nc) as tc:
        with (
            tc.tile_pool(name="const", bufs=1) as const,
            tc.tile_pool(name="xin", bufs=4) as xin,
            tc.tile_pool(name="probs", bufs=6) as probs,
            tc.tile_pool(name="norm", bufs=6) as norm,
            tc.tile_pool(name="ot", bufs=6) as otp,
            tc.tile_pool(name="osb", bufs=3) as osbp,
            tc.tile_pool(name="fin", bufs=4) as fin,
            tc.tile_pool(name="ps1", bufs=1, space="PSUM") as ps1,
            tc.tile_pool(name="ps2", bufs=2, space="PSUM") as ps2,
            tc.tile_pool(name="dram", bufs=1, space="DRAM") as dram,
        ):
            # ---- persistent SBUF state ----
            w_sb = const.tile([128, KC * WCOLS], BF16)
            wt3 = wt.rearrange("(kc p) j -> p kc j", p=128)
            for hk in range(2):
                nc.sync.dma_start(
                    w_sb[:, hk * 4 * WCOLS : (hk + 1) * 4 * WCOLS].rearrange(
                        "p (kc j) -> p kc j", kc=4
                    ),
                    wt3[:, hk * 4 : (hk + 1) * 4, :],
                )
            # wo/bias loads are deferred until after batch 0's x tiles are on
            # the sync queue -- they aren't needed until the first outproj
            wo_sb = const.tile([128, KC * D], BF16)
            b_row = const.tile([1, D], FP32)
            bias_sb = const.tile([128, D], FP32)

            def emit_wo_load():
                nc.sync.dma_start(
                    wo_sb[:].rearrange("p (kc n) -> p kc n", kc=KC),
                    wo.rearrange("(kc p) n -> p kc n", p=128),
                )
                nc.sync.dma_start(b_row[:], bias[:])
                nc.gpsimd.partition_broadcast(bias_sb[:], b_row[:])

            q_sb = const.tile([128, T], BF16)  # [2 heads x 64, tokens] scaled
            k_sb = const.tile([128, T], BF16)
            # V token-major: [128 tok-in-chunk, (global chunk, head) x 65]
            v_sb = const.tile([128, (T // 128) * HL * 65], BF16)
            v3 = v_sb[:].rearrange("p (blk e) -> p blk e", e=65)
            nc.vector.memset(v3[:, :, 64:65], 1.0)

            a2a_in = {}
            a2a_out = {}
            for b in range(B - 1):
                a2a_in[b] = dram.tile(
                    [N_CORES, HL * HD, TPB], BF16, name=f"a2a_in{b}"
                )
                a2a_out[b] = dram.tile(
                    [N_CORES, HL * HD, TPB], BF16, name=f"a2a_out{b}"
                )
            # last batch: a half piece (qt0+qt1) plus two quarter pieces (qt2,
            # qt3) so the tail collective covers only qt3 (measured faster
            # than a qt2+qt3 half despite the CC serialization)
            a2a_in3 = {}
            a2a_out3 = {}
            a2a_in3[0] = dram.tile([N_CORES, HL * HD, 128], BF16, name="a2a_in3_0")
            a2a_out3[0] = dram.tile([N_CORES, HL * HD, 128], BF16, name="a2a_out3_0")
            for qp in (2, 3):
                a2a_in3[qp] = dram.tile(
                    [N_CORES, HL * HD, 64], BF16, name=f"a2a_in3_{qp}"
                )
                a2a_out3[qp] = dram.tile(
                    [N_CORES, HL * HD, 64], BF16, name=f"a2a_out3_{qp}"
                )

            def emit_a2a(b):
                nc.gpsimd.collective_compute(
                    "AllToAll",
                    mybir.AluOpType.bypass,
                    replica_groups=[list(range(N_CORES))],
                    ins=[a2a_in[b].opt()],
                    outs=[a2a_out[b].opt()],
                )

            staged_osb = {}
            staged_osbh = {}

            def emit_outproj_dma(b):
                o_sb = osbp.tile([128, N_CORES * TPB], BF16, tag="osb", name="o_sb")
                for i in range(N_CORES):
                    nc.sync.dma_start(
                        o_sb[:, i * TPB : (i + 1) * TPB], a2a_out[b][i, :, :]
                    )
                return o_sb

            def emit_x_load(tt):
                # two half-tile DMAs: [128, 4, 512] each
                x_t = xin.tile([128, KC * TN], BF16, tag="xt", name="x_t")
                for hk in range(2):
                    nc.sync.dma_start(
                        x_t[:, hk * 4 * TN : (hk + 1) * 4 * TN].rearrange(
                            "p (kc j) -> p kc j", kc=4
                        ),
                        xt3[:, hk * 4 : (hk + 1) * 4, tt * TN : (tt + 1) * TN],
                    )
                return x_t

            def emit_qk_piece(x_t, tt, m, ydst):
                # ydst: [128, 512] psum view; m=0 -> Q (scaled), m=1 -> K
                for kc in range(KC):
                    nc.tensor.matmul(
                        ydst[:, :],
                        lhsT=w_sb[
                            :, kc * WCOLS + m * 128 : kc * WCOLS + (m + 1) * 128
                        ],
                        rhs=x_t[:, kc * TN : (kc + 1) * TN],
                        start=(kc == 0),
                        stop=(kc == KC - 1),
                    )
                if m == 0:
                    nc.vector.tensor_scalar_mul(
                        q_sb[:, tt * TN : (tt + 1) * TN], ydst[:, :], SCALE
                    )
                else:
                    nc.vector.tensor_copy(
                        k_sb[:, tt * TN : (tt + 1) * TN], ydst[:, :]
                    )

            def emit_v_piece(x_t, tt, ydst):
                # V natural layout: 4 token subtiles share one PSUM bank;
                # start=True clears has_written flags bank-wide, so chain
                # ordering deps across the accumulation groups
                prev = None
                for s in range(4):
                    for kc in range(KC):
                        mm = nc.tensor.matmul(
                            ydst[:, s * 128 : (s + 1) * 128],
                            lhsT=x_t[
                                :, kc * TN + s * 128 : kc * TN + (s + 1) * 128
                            ],
                            rhs=w_sb[:, kc * WCOLS + 256 : kc * WCOLS + WCOLS],
                            start=(kc == 0),
                            stop=(kc == KC - 1),
                        )
                        if prev is not None:
                            add_dep_helper(
                                mm.ins, prev.ins, sync=False,
                                reason="bank flag-clear order",
                            )
                        prev = mm
                nc.vector.tensor_copy(
                    v3[:, (tt * 4) * HL : (tt * 4 + 4) * HL, 0:64],
                    ydst[:, :]
                    .rearrange("p (s hd) -> p s hd", s=4)
                    .rearrange("p s (h d) -> p (s h) d", h=HL),
                )

            pending = []  # (group, p_t, pv, src_b) with S+exp emitted, PV not

            for b in range(B):
                slot_list = [(kc, h) for kc in range(KT) for h in range(HL)]
                groups = []
                off = 0
                for w in WIDTHS:
                    groups.append(slot_list[off : off + w])
                    off += w

                def emit_pv_flush():
                    group, p_t, pv, bsrc = pending.pop(0)
                    for i, (kc, h) in enumerate(group):
                        gc = bsrc * KT + kc
                        nc.tensor.matmul(
                            pv[h][0:65, :],
                            lhsT=v3[:, gc * HL + h, :],
                            rhs=p_t[:, i * 512 : (i + 1) * 512],
                            start=(kc == 0),
                            stop=(kc == KT - 1),
                        )

                def emit_group(group, pv, qt, b=b, flush=True):
                    # S matmuls + exp for this group; PV emitted 2 groups
                    # later (via pending) so the in-order PE queue never
                    # head-stalls on the exp of its own group
                    q_off = b * NTOK + qt * TN
                    width = len(group) * 512
                    if len(group) == 3:
                        s_t = ps1.tile([128, 1536], FP32, tag="e3", name="s_t3")
                    else:
                        s_t = ps1.tile([128, 1024], FP32, tag="e2", name="s_t2")
                    for i, (kc, h) in enumerate(group):
                        nc.tensor.matmul(
                            s_t[:, i * 512 : (i + 1) * 512],
                            lhsT=k_sb[
                                h * 64 : (h + 1) * 64,
                                b * NTOK + kc * 128 : b * NTOK + (kc + 1) * 128,
                            ],
                            rhs=q_sb[h * 64 : (h + 1) * 64, q_off : q_off + TN],
                            start=True,
                            stop=True,
                        )
                    p_t = probs.tile([128, 1536], BF16, tag="p", name="p_t")
                    nc.scalar.activation(p_t[:, 0:width], s_t[:, 0:width], AF.Exp)
                    pending.append((group, p_t, pv, b))
                    if flush:
                        while len(pending) > 2:
                            emit_pv_flush()

                def finish_qt(pv, qt, b=b):
                    # both heads' chains emitted in lockstep so their queue
                    # slots interleave
                    o_c = [
                        norm.tile([65, 512], FP32, tag="oc", name="o_c")
                        for _ in range(HL)
                    ]
                    for h in range(HL):
                        nc.vector.tensor_copy(o_c[h][:], pv[h][0:65, :])
                    rs = [
                        norm.tile([128, 4], FP32, tag="rs", name="rs")
                        for _ in range(HL)
                    ]
                    for h in range(HL):
                        nc.sync.dma_start(rs[h][:], o_c[h][64:65, :])
                    rr = [
                        norm.tile([128, 4], FP32, tag="rr", name="rr")
                        for _ in range(HL)
                    ]
                    for h in range(HL):
                        nc.vector.reciprocal(rr[h][:], rs[h][:])
                    rec = [
                        norm.tile([1, 512], FP32, tag="rec", name="rec")
                        for _ in range(HL)
                    ]
                    for h in range(HL):
                        nc.sync.dma_start(rec[h][:], rr[h][:])
                    bc = [
                        norm.tile([64, 512], FP32, tag="bc", name="bc")
                        for _ in range(HL)
                    ]
                    for h in range(HL):
                        nc.gpsimd.partition_broadcast(bc[h][:], rec[h][:])
                    for h in range(HL):
                        o_t = otp.tile([64, 512], BF16, tag="o", name="o_t")
                        nc.vector.tensor_mul(o_t[:], o_c[h][0:64, :], bc[h][:])
                        if b < B - 1:
                            nc.sync.dma_start(
                                a2a_in[b][
                                    2 * qt : 2 * qt + 2, h * 64 : (h + 1) * 64, :
                                ].rearrange("j p e -> p j e"),
                                o_t[:].rearrange("p (j e) -> p j e", j=2),
                            )
                        elif qt < 2:
                            nc.sync.dma_start(
                                a2a_in3[0][
                                    qt * 4 : qt * 4 + 4, h * 64 : (h + 1) * 64, :
                                ].rearrange("j p e -> p j e"),
                                o_t[:].rearrange("p (j e) -> p j e", j=4),
                            )
                        else:
                            nc.sync.dma_start(
                                a2a_in3[qt][
                                    :, h * 64 : (h + 1) * 64, :
                                ].rearrange("j p e -> p j e"),
                                o_t[:].rearrange("p (j e) -> p j e", j=8),
                            )

                # ---- filler units: QKV(b+1) + outproj(b-1), run between
                # attention groups so the PE stays busy under the exp-bound
                # steady state ----
                filler_q = []
                filler_late = []
                if b < B - 1:
                    bb = b + 1
                    xts_n = {}

                    def mk_x(t, bb=bb, xts_n=xts_n):
                        def u():
                            xts_n[t] = emit_x_load(4 * bb + t)
                        return u

                    def mk_qk(t, m, bb=bb, xts_n=xts_n):
                        def u():
                            y = ps1.tile(
                                [128, 512], FP32, tag="fill", name="y_qk"
                            )
                            emit_qk_piece(xts_n[t], 4 * bb + t, m, y[:, :])
                        return u

                    def mk_v(t, bb=bb, xts_n=xts_n):
                        def u():
                            y = ps1.tile(
                                [128, 512], FP32, tag="fill", name="y_v"
                            )
                            emit_v_piece(xts_n[t], 4 * bb + t, y[:, :])
                        return u

                    filler_q = [
                        mk_x(0), mk_x(1),
                        mk_qk(0, 0), mk_qk(0, 1), mk_v(0),
                        mk_x(2),
                        mk_qk(1, 0), mk_qk(1, 1), mk_v(1),
                        mk_x(3),
                        mk_qk(2, 0), mk_qk(2, 1), mk_v(2),
                        mk_qk(3, 0), mk_qk(3, 1), mk_v(3),
                    ]
                if b >= 1:

                    def mk_op(m, nh, b=b):
                        def u():
                            o_sb = staged_osb[b - 1]
                            y = ps1.tile(
                                [128, 512], FP32, tag="fill", name="y_op"
                            )
                            for i in range(N_CORES):
                                nc.tensor.matmul(
                                    y[:, :],
                                    lhsT=o_sb[
                                        :, i * TPB + m * 128 : i * TPB + (m + 1) * 128
                                    ],
                                    rhs=wo_sb[
                                        :, i * D + nh * 512 : i * D + nh * 512 + 512
                                    ],
                                    start=(i == 0),
                                    stop=(i == N_CORES - 1),
                                )
                            out_t = fin.tile(
                                [128, 512], FP32, tag="outt", name="out_t"
                            )
                            nc.vector.tensor_add(
                                out_t[:], y[:, :],
                                bias_sb[:, nh * 512 : (nh + 1) * 512],
                            )
                            nc.sync.dma_start(
                                out[
                                    (b - 1) * TPB + m * 128 : (b - 1) * TPB
                                    + (m + 1) * 128,
                                    nh * 512 : (nh + 1) * 512,
                                ],
                                out_t[:],
                            )
                        return u

                    filler_late = [
                        mk_op(0, 0), mk_op(0, 1), mk_op(1, 0), mk_op(1, 1)
                    ]
                    if b == B - 1:
                        # the final m=0 half-batch out-proj also runs as qt3
                        # filler (its o_sbh is staged at the qt3 gi==1 hook,
                        # well before these units pop at gi>=9)
                        def mk_m0(nh, b=b):
                            def u():
                                o_sbh = staged_osbh[0]
                                y = ps1.tile(
                                    [128, 512], FP32, tag="fill", name="y_m0"
                                )
                                for i in range(N_CORES):
                                    nc.tensor.matmul(
                                        y[:, :],
                                        lhsT=o_sbh[:, i * 128 : (i + 1) * 128],
                                        rhs=wo_sb[
                                            :,
                                            i * D + nh * 512 : i * D + nh * 512 + 512,
                                        ],
                                        start=(i == 0),
                                        stop=(i == N_CORES - 1),
                                    )
                                out_t = fin.tile(
                                    [128, 512], FP32, tag="outt", name="out_m0"
                                )
                                nc.vector.tensor_add(
                                    out_t[:], y[:, :],
                                    bias_sb[:, nh * 512 : (nh + 1) * 512],
                                )
                                nc.sync.dma_start(
                                    out[
                                        (B - 1) * TPB : (B - 1) * TPB + 128,
                                        nh * 512 : (nh + 1) * 512,
                                    ],
                                    out_t[:],
                                )
                            return u

                        filler_late += [mk_m0(0), mk_m0(1)]

                # ---- batch 0 prologue: QKV on the e3/e2+fill rotation,
                # interleaved with qt0 attention as K/V chunks land ----
                if b == 0:
                    pv0 = [
                        ps2.tile([128, 512], FP32, tag="pv", name=f"pv{h}")
                        for h in range(HL)
                    ]
                    g_next = 0
                    for i in range(4):
                        x_t = emit_x_load(i)
                        if i % 2 == 0:
                            y = ps1.tile([128, 1536], FP32, tag="e3", name="y3")
                            qy, ky, vy = (
                                y[:, 0:512], y[:, 512:1024], y[:, 1024:1536]
                            )
                        else:
                            y2 = ps1.tile([128, 1024], FP32, tag="e2", name="y2")
                            yf = ps1.tile([128, 512], FP32, tag="fill", name="yf")
                            qy, ky, vy = y2[:, 0:512], y2[:, 512:1024], yf[:, :]
                        emit_qk_piece(x_t, i, 0, qy)
                        emit_qk_piece(x_t, i, 1, ky)
                        emit_v_piece(x_t, i, vy)
                        if i == 1:
                            emit_wo_load()
                        avail = 4 * (i + 1)
                        while g_next < len(groups) and all(
                            kc < avail for kc, _ in groups[g_next]
                        ):
                            emit_group(groups[g_next], pv0, 0, flush=False)
                            g_next += 1
                        while len(pending) > 2:
                            emit_pv_flush()
                    fin_q = [(finish_qt, pv0, 0)]

                # ---- attention q tiles ----
                for qt in range(1 if b == 0 else 0, NTOK // TN):
                    pv = [
                        ps2.tile([128, 512], FP32, tag="pv", name=f"pv{h}")
                        for h in range(HL)
                    ]
                    for gi, g in enumerate(groups):
                        emit_group(g, pv, qt)
                        if gi == 1:
                            if fin_q:
                                fn, pv_, qt_ = fin_q.pop(0)
                                fn(pv_, qt_)
                                if qt == 0:
                                    # popped the previous batch's qt3; its
                                    # A2A can fire now
                                    emit_a2a(b - 1)
                                if b == B - 1 and qt == 2:
                                    # qt0+qt1 of the last batch are complete
                                    nc.gpsimd.collective_compute(
                                        "AllToAll",
                                        mybir.AluOpType.bypass,
                                        replica_groups=[list(range(N_CORES))],
                                        ins=[a2a_in3[0].opt()],
                                        outs=[a2a_out3[0].opt()],
                                    )
                            if qt == 2 and b >= 1:
                                # A2A(b-1) is long done; stage its o_sb here
                                # so the DMAs never head-block the sync queue
                                staged_osb[b - 1] = emit_outproj_dma(b - 1)
                            if qt == 3 and b == B - 1:
                                # qt2's quarter A2A + stage the m=0 half
                                nc.gpsimd.collective_compute(
                                    "AllToAll",
                                    mybir.AluOpType.bypass,
                                    replica_groups=[list(range(N_CORES))],
                                    ins=[a2a_in3[2].opt()],
                                    outs=[a2a_out3[2].opt()],
                                )
                                o_sbh0 = osbp.tile(
                                    [128, N_CORES * 128], BF16,
                                    tag="osb", name="o_sbh0",
                                )
                                for i in range(N_CORES):
                                    nc.sync.dma_start(
                                        o_sbh0[:, i * 128 : (i + 1) * 128],
                                        a2a_out3[0][i, :, :],
                                    )
                                staged_osbh[0] = o_sbh0
                        if gi % 2 == 1:
                            if filler_q:
                                filler_q.pop(0)()
                            elif qt >= 3 and filler_late:
                                filler_late.pop(0)()
                    fin_q.append((finish_qt, pv, qt))

                while filler_q:
                    filler_q.pop(0)()
                while filler_late:
                    filler_late.pop(0)()
                # for b < B-1 the qt3 pending PVs and finish chain roll
                # into the next batch's qt0 pipeline (no boundary DVE burst)
                if b == B - 1:
                    while pending:
                        emit_pv_flush()
                    for fn, pv_, qt_ in fin_q:
                        fn(pv_, qt_)
                    fin_q = []

            # ---- tail: qt3 quarter collective, then the stitched m=1 ----
            nc.gpsimd.collective_compute(
                "AllToAll",
                mybir.AluOpType.bypass,
                replica_groups=[list(range(N_CORES))],
                ins=[a2a_in3[3].opt()],
                outs=[a2a_out3[3].opt()],
            )
            o_sbh = osbp.tile(
                [128, N_CORES * 128], BF16, tag="osb", name="o_sbh1"
            )
            for i in range(N_CORES):
                nc.sync.dma_start(
                    o_sbh[:, i * 128 : i * 128 + 64], a2a_out3[2][i, :, :]
                )
            for i in range(N_CORES):
                nc.sync.dma_start(
                    o_sbh[:, i * 128 + 64 : (i + 1) * 128], a2a_out3[3][i, :, :]
                )
            o_ps = [
                ps2.tile([128, 512], FP32, tag="pv", name=f"o_ps{nh}")
                for nh in range(2)
            ]
            for i in range(N_CORES):
                for nh in range(2):
                    nc.tensor.matmul(
                        o_ps[nh][:, :],
                        lhsT=o_sbh[:, i * 128 : (i + 1) * 128],
                        rhs=wo_sb[:, i * D + nh * 512 : i * D + nh * 512 + 512],
                        start=(i == 0),
                        stop=(i == N_CORES - 1),
                    )
            out_t = fin.tile([128, D], FP32, tag="outt2", name="out_t2")
            for nh in range(2):
                nc.vector.tensor_add(
                    out_t[:, nh * 512 : (nh + 1) * 512],
                    o_ps[nh][:, :],
                    bias_sb[:, nh * 512 : (nh + 1) * 512],
                )
            nc.sync.dma_start(
                out[(B - 1) * TPB + 128 : (B - 1) * TPB + 256, :],
                out_t[:],
            )

    nc.compile()
    return nc


_NC_CACHE = None


def _get_nc():
    global _NC_CACHE
    if _NC_CACHE is None:
        _NC_CACHE = build_nc()
    return _NC_CACHE


def make_in_maps(x, w_qkv, w_out, b_out):
    x = np.asarray(x, dtype=np.float32)
    w_qkv = np.asarray(w_qkv, dtype=np.float32)
    w_out = np.asarray(w_out, dtype=np.float32)
    b_out = np.asarray(b_out, dtype=np.float32)

    xt_np = np.ascontiguousarray(x.reshape(T, D).T).astype(ml_dtypes.bfloat16)
    wo_np = np.ascontiguousarray(w_out.T).astype(ml_dtypes.bfloat16)
    b_np = np.ascontiguousarray(b_out.reshape(1, D))

    in_maps = []
    for c in range(N_CORES):
        rows = []
        for sec in range(3):  # q, k, v sections of w_qkv
            for hh in range(HL):
                h = HL * c + hh
                rows.append(w_qkv[sec * D + h * HD : sec * D + (h + 1) * HD, :])
        wt_np = np.ascontiguousarray(np.concatenate(rows, 0).T).astype(
            ml_dtypes.bfloat16
        )  # (1024, 384)
        in_maps.append({"xt": xt_np, "wt": wt_np, "wo": wo_np, "bias": b_np})
    return in_maps


def unshard(results):
    # core j out rows: batches 0-2: r = b*256+u -> token b*2048 + j*256 + u;
    # batch 3: m0 rows cover qt0+qt1 (token 6144 + j*128 + u), m1 rows:
    # u<64 -> qt2 token 7168 + j*64 + u, u>=64 -> qt3 token 7680 + j*64
    full = np.empty((T, D), np.float32)
    for j in range(N_CORES):
        o = np.asarray(results[j]["out"], dtype=np.float32)
        for b in range(B - 1):
            full[b * NTOK + j * TPB : b * NTOK + (j + 1) * TPB] = o[
                b * TPB : (b + 1) * TPB
            ]
        r0 = (B - 1) * TPB
        full[6144 + j * 128 : 6144 + j * 128 + 128] = o[r0 : r0 + 128]
        full[7168 + j * 64 : 7168 + j * 64 + 64] = o[r0 + 128 : r0 + 192]
        full[7680 + j * 64 : 7680 + j * 64 + 64] = o[r0 + 192 : r0 + 256]
    return full.reshape(B, NTOK, D)


def kernel(x, w_qkv, w_out, b_out, _trace=False, _tmpdir=None):
    in_maps = make_in_maps(x, w_qkv, w_out, b_out)
    nc = _get_nc()
    res = bass_utils.run_bass_kernel_spmd(
        nc, in_maps, core_ids=list(range(N_CORES)), trace=_trace, tmpdir=_tmpdir
    )
    kernel.last_result = res
    return unshard([res.results[j] for j in range(N_CORES)])
